# revision 1
# baseline (speedup 1.0000x reference)
"""CPMAnt attention kernel for Trainium2, 8-core tensor-parallel over heads.

Math (per reference):
    q = hq @ Wq; k = hkv @ Wk; v = hkv @ Wv           (heads split col-wise)
    score = (q k^T) / sqrt(dh) + position_bias, masked
    probs = softmax(score);  out = (probs @ v) @ Wo    (Wo split row-wise)

Sharding: core c owns heads [HPC*c, HPC*(c+1)): Wq/Wk/Wv column slices,
Wo row slice, position_bias head slice. Each core returns a partial
output (its heads' contribution through Wo); the host sums the 8
partials (the "all-reduce" of the row-parallel projection).

Device layout notes:
 - hidden states are shipped pre-transposed (X^T, [D, B*S]) so the
   contraction dim D lands on SBUF partitions for the projections.
 - Q^T/K^T are produced in [dh, seq] layout (lhsT = W chunk), V in
   [seq, dh] (lhsT = X^T chunk), which feeds QK^T and PV matmuls with
   only probs needing a runtime PE transpose. V stays resident in SBUF;
   Q^T/K^T round-trip through DRAM scratch (SBUF pressure).
 - scale 1/sqrt(dh) is folded into Wq on the host; mask is folded into
   the position-bias slice as a -1e30 addend on the host (bf16).
 - bias add rides on the PE: score_psum += I^T @ pb_tile (identity
   matmul accumulating onto the QK^T bank).
 - softmax skips the max-subtraction pass (scores are O(10) here, exp
   is safe in fp32) and gets the row sum for free from the ACT
   accumulator during exp; the 1/sum scale is applied to probs rows.
 - all big matmuls run as float32r (full-rate fp32 mode at N>=256);
   every matmul operand is produced with dtype float32r end-to-end
   (walrus BIR verifier requirement).
 - W loads are split into 8 chunked DMAs to spread across queues.
"""

import contextlib
import sys

sys.path.insert(0, "/opt/trn_rl_repo")

import ml_dtypes
import numpy as np

import concourse.bacc as bacc
import concourse.mybir as mybir
import concourse.tile as tile
from concourse.masks import make_identity

F32 = mybir.dt.float32
F32R = mybir.dt.float32r
BF16 = mybir.dt.bfloat16
AF = mybir.ActivationFunctionType
P = 128
DH = 128
NEG = -1.0e30


def _r(ap):
    return ap.bitcast(F32R)


def build_program(B, S, D, HPC, nkb_tab, n_cores=8, reps=1):
    """Emit the per-core SPMD program (identical on every core).

    nkb_tab[b][qt]: number of KB-wide score blocks to compute for the
    128-row q-tile qt of batch b (blocks past the last unmasked key are
    skipped entirely; skipped probs are zero by construction).
    reps>1 wraps the body in a For_i loop (benchmarking only).
    """
    WPC = HPC * DH            # per-core projection width
    BS = B * S
    KB = min(512, S)          # score/key block width
    ST = 512                  # phase-A seq tile
    DCOL = min(512, D)        # phase-C output col block
    n_d = D // P
    n_st = BS // ST
    n_m = WPC // P
    n_sub = ST // P
    QSUP = min(512, S)        # q super-tile for transpose+PV batching
    n_qsup = S // QSUP
    n_qts = QSUP // P
    n_dcol = D // DCOL
    nkb_max = max(max(r) for r in nkb_tab)
    NCH = 8 if n_d % 8 == 0 else 1   # W-load DMA chunking

    nc = bacc.Bacc("TRN2", target_bir_lowering=False, debug=False,
                   num_devices=n_cores)
    xqT = nc.dram_tensor("xqT", [D, BS], F32R, kind="ExternalInput").ap()
    xkT = nc.dram_tensor("xkT", [D, BS], F32R, kind="ExternalInput").ap()
    wq = nc.dram_tensor("wq", [D, WPC], F32R, kind="ExternalInput").ap()
    wk = nc.dram_tensor("wk", [D, WPC], F32R, kind="ExternalInput").ap()
    wv = nc.dram_tensor("wv", [D, WPC], F32R, kind="ExternalInput").ap()
    wo = nc.dram_tensor("wo", [WPC, D], F32R, kind="ExternalInput").ap()
    pbm = nc.dram_tensor("pbm", [B, HPC, S, S], BF16,
                         kind="ExternalInput").ap()
    out = nc.dram_tensor("out", [BS, D], F32, kind="ExternalOutput").ap()

    with tile.TileContext(nc, pool_alloc_mode="queue") as tc, \
            contextlib.ExitStack() as es:
        dpool = es.enter_context(tc.tile_pool(name="dram", bufs=1,
                                              space="DRAM"))
        cpool = es.enter_context(tc.tile_pool(name="const", bufs=1))
        ctx_pool = es.enter_context(tc.tile_pool(name="ctx", bufs=1))

        qt_s = dpool.tile([WPC, BS], F32R, tag="qt_s")
        kt_s = dpool.tile([WPC, BS], F32R, tag="kt_s")
        # V stays resident in SBUF through phase B ([p, seq-tile, d])
        v_sb = ctx_pool.tile([P, BS // P, WPC], F32R, tag="v_sb")

        ident_f = cpool.tile([P, P], F32, tag="ident_f")
        make_identity(nc, ident_f[:])
        # DVE cast-copies so the verifier sees engine-rounded producers.
        ident = cpool.tile([P, P], F32R, tag="ident")
        nc.vector.tensor_copy(ident[:], ident_f[:])
        ident_bf = cpool.tile([P, P], BF16, tag="ident_bf")
        nc.vector.tensor_copy(ident_bf[:], ident_f[:])

        rep_cm = tc.For_i(0, reps, 1) if reps > 1 else contextlib.nullcontext()
        with rep_cm:
            # ---------- Phase A1: Q^T = (X Wq)^T ----------
            with (
                tc.tile_pool(name="a1w", bufs=1) as wpool,
                tc.tile_pool(name="a1x", bufs=12) as xpool,
                tc.tile_pool(name="a1s", bufs=4) as spool,
                tc.tile_pool(name="a1p", bufs=2, space="PSUM") as psa,
            ):
                wq_sb = wpool.tile([P, n_d, WPC], F32R, tag="wq_sb")
                wq_r = wq.rearrange("(a p) c -> p a c", p=P)
                for c in range(NCH):
                    sl = slice(c * n_d // NCH, (c + 1) * n_d // NCH)
                    nc.sync.dma_start(wq_sb[:, sl, :], wq_r[:, sl, :])
                for n in range(n_st):
                    ps = psa.tile([P, n_m, 512], F32, tag="ps_a")
                    for d in range(n_d):
                        xt = xpool.tile([P, ST], F32R, tag="xt")
                        nc.sync.dma_start(
                            xt[:], xqT[d * P:(d + 1) * P, n * ST:(n + 1) * ST])
                        for m in range(n_m):
                            nc.tensor.matmul(
                                ps[:, m, :ST],
                                _r(wq_sb[:, d, m * P:(m + 1) * P]),
                                _r(xt[:]),
                                start=(d == 0), stop=(d == n_d - 1))
                    for m in range(n_m):
                        st = spool.tile([P, ST], F32R, tag="st")
                        nc.scalar.copy(st[:], ps[:, m, :ST])
                        nc.sync.dma_start(
                            qt_s[m * P:(m + 1) * P, n * ST:(n + 1) * ST],
                            st[:])

            # ---------- Phase A2: K^T and V (one pass over X_kv^T) ------
            with (
                tc.tile_pool(name="a2w", bufs=1) as wpool2,
                tc.tile_pool(name="a2x", bufs=4) as xpool2,
                tc.tile_pool(name="a2s", bufs=2) as spool2,
                tc.tile_pool(name="a2pk", bufs=1, space="PSUM") as psk_pool,
                tc.tile_pool(name="a2pv", bufs=1, space="PSUM") as psv_pool,
            ):
                wk_sb = wpool2.tile([P, n_d, WPC], F32R, tag="wk_sb")
                wk_r = wk.rearrange("(a p) c -> p a c", p=P)
                for c in range(NCH):
                    sl = slice(c * n_d // NCH, (c + 1) * n_d // NCH)
                    nc.sync.dma_start(wk_sb[:, sl, :], wk_r[:, sl, :])
                wv_sb = wpool2.tile([P, n_d, WPC], F32R, tag="wv_sb")
                wv_r = wv.rearrange("(a p) c -> p a c", p=P)
                for c in range(NCH):
                    sl = slice(c * n_d // NCH, (c + 1) * n_d // NCH)
                    nc.sync.dma_start(wv_sb[:, sl, :], wv_r[:, sl, :])
                for n in range(n_st):
                    psk = psk_pool.tile([P, n_m, 512], F32, tag="ps_k")
                    psv = psv_pool.tile([P, n_sub, 512], F32, tag="ps_v")
                    for d in range(n_d):
                        xt = xpool2.tile([P, ST], F32R, tag="xt2")
                        nc.sync.dma_start(
                            xt[:], xkT[d * P:(d + 1) * P, n * ST:(n + 1) * ST])
                        for m in range(n_m):
                            nc.tensor.matmul(
                                psk[:, m, :ST],
                                _r(wk_sb[:, d, m * P:(m + 1) * P]),
                                _r(xt[:]),
                                start=(d == 0), stop=(d == n_d - 1))
                        for s2 in range(n_sub):
                            nc.tensor.matmul(
                                psv[:, s2, :WPC],
                                _r(xt[:, s2 * P:(s2 + 1) * P]),
                                _r(wv_sb[:, d, :]),
                                start=(d == 0), stop=(d == n_d - 1))
                    for m in range(n_m):
                        st = spool2.tile([P, ST], F32R, tag="stk")
                        nc.scalar.copy(st[:], psk[:, m, :ST])
                        nc.sync.dma_start(
                            kt_s[m * P:(m + 1) * P, n * ST:(n + 1) * ST],
                            st[:])
                    for s2 in range(n_sub):
                        nc.vector.tensor_copy(
                            v_sb[:, n * n_sub + s2, :WPC],
                            psv[:, s2, :WPC])
            # ---------- Phase B: attention per (b, h) ----------
            ctx_tiles = {}
            with (
                tc.tile_pool(name="bh", bufs=2) as bh_pool,
                tc.tile_pool(name="pb", bufs=7) as pb_pool,
                tc.tile_pool(name="probs", bufs=8) as probs_pool,
                tc.tile_pool(name="pt", bufs=4) as pt_pool,
                tc.tile_pool(name="rsum", bufs=8) as rsum_pool,
                tc.tile_pool(name="bps", bufs=2, space="PSUM") as psum_s,
                tc.tile_pool(name="bpt", bufs=2, space="PSUM") as psum_t,
                tc.tile_pool(name="bpc", bufs=2, space="PSUM") as psum_c,
            ):
                for b in range(B):
                    for h in range(HPC):
                        qth = bh_pool.tile([P, S], F32R, tag="qth")
                        nc.sync.dma_start(
                            qth[:],
                            qt_s[h * P:(h + 1) * P, b * S:(b + 1) * S])
                        kth = bh_pool.tile([P, S], F32R, tag="kth")
                        nc.sync.dma_start(
                            kth[:],
                            kt_s[h * P:(h + 1) * P, b * S:(b + 1) * S])
                        ctx_t = ctx_pool.tile([P, S], F32R,
                                              tag=f"ctx_{b}_{h}")
                        ctx_tiles[(b, h)] = ctx_t

                        for sup in range(n_qsup):
                            kmax_sup = max(
                                nkb_tab[b][sup * n_qts + qt]
                                for qt in range(n_qts)) * KB
                            nj = kmax_sup // P
                            probs_list = []
                            for qt in range(n_qts):
                                gqt = sup * n_qts + qt
                                nkb = nkb_tab[b][gqt]
                                q0 = gqt * P
                                pss = psum_s.tile([P, nkb_max, KB], F32,
                                                  tag="ps_s")
                                probs = probs_pool.tile([P, S], F32R,
                                                        tag="probs")
                                sums = rsum_pool.tile([P, max(2, nkb_max)],
                                                      F32, tag="sums")
                                for kb in range(nkb):
                                    nc.tensor.matmul(
                                        pss[:, kb, :],
                                        _r(qth[:, q0:q0 + P]),
                                        _r(kth[:, kb * KB:(kb + 1) * KB]),
                                        start=True, stop=False)
                                    pb_t = pb_pool.tile([P, KB], BF16,
                                                        tag="pb")
                                    nc.sync.dma_start(
                                        pb_t[:],
                                        pbm[b, h, q0:q0 + P,
                                            kb * KB:(kb + 1) * KB])
                                    nc.tensor.matmul(
                                        pss[:, kb, :], ident_bf[:],
                                        pb_t[:], start=False, stop=True)
                                    nc.scalar.activation(
                                        probs[:, kb * KB:(kb + 1) * KB],
                                        pss[:, kb, :], AF.Exp,
                                        accum_out=sums[:, kb:kb + 1])
                                if nkb * KB < kmax_sup:
                                    nc.gpsimd.memset(
                                        probs[:, nkb * KB:kmax_sup], 0.0)
                                recip = rsum_pool.tile([P, 1], F32,
                                                       tag="recip")
                                if nkb == 1:
                                    nc.vector.reciprocal(recip[:],
                                                         sums[:, 0:1])
                                else:
                                    tot = rsum_pool.tile([P, 1], F32,
                                                         tag="tot")
                                    nc.vector.tensor_add(
                                        tot[:], sums[:, 0:1], sums[:, 1:2])
                                    for kb in range(2, nkb):
                                        nc.vector.tensor_add(
                                            tot[:], tot[:],
                                            sums[:, kb:kb + 1])
                                    nc.vector.reciprocal(recip[:], tot[:])
                                nc.vector.tensor_scalar_mul(
                                    probs[:, :nkb * KB],
                                    probs[:, :nkb * KB], recip[:])
                                probs_list.append(probs)

                            ps_ctx = psum_c.tile([P, QSUP], F32,
                                                 tag="ps_ctx")
                            for j in range(nj):
                                # first q-tile whose computed k-range covers
                                # block j; earlier q-tiles have zero probs
                                # there and can skip transpose+PV entirely.
                                vq = 0
                                while (vq < n_qts and
                                       nkb_tab[b][sup * n_qts + vq] * KB
                                       <= j * P):
                                    vq += 1
                                if vq >= n_qts:
                                    continue
                                # keep PV free-dim >=256 (f32r full rate)
                                if (n_qts - vq) * P < 256:
                                    vq = max(0, n_qts - 256 // P)
                                ps_t = psum_t.tile([P, QSUP], F32,
                                                   tag="ps_t")
                                for qt in range(vq, n_qts):
                                    nc.tensor.transpose(
                                        _r(ps_t[:, qt * P:(qt + 1) * P]),
                                        probs_list[qt][:, j * P:(j + 1) * P],
                                        ident[:])
                                pT = pt_pool.tile([P, QSUP], F32R, tag="pT")
                                nc.vector.tensor_copy(pT[:, vq * P:],
                                                      ps_t[:, vq * P:])
                                nc.tensor.matmul(
                                    ps_ctx[:, vq * P:],
                                    _r(v_sb[:, b * (S // P) + j,
                                            h * DH:(h + 1) * DH]),
                                    _r(pT[:, vq * P:]),
                                    start=(j == 0), stop=(j == nj - 1))
                            nc.vector.tensor_copy(
                                ctx_t[:, sup * QSUP:(sup + 1) * QSUP],
                                ps_ctx[:])

            # ---------- Phase C: out = ctx @ Wo (partial over heads) -----
            with (
                tc.tile_pool(name="co", bufs=4) as opool,
                tc.tile_pool(name="cw", bufs=1) as wopool,
                tc.tile_pool(name="cp", bufs=4, space="PSUM") as psum_o,
            ):
                wo_sb = wopool.tile([P, HPC, D], F32R, tag="wo_sb")
                wo_r = wo.rearrange("(h p) d -> p h d", p=P)
                for c in range(8):
                    sl = slice(c * D // 8, (c + 1) * D // 8)
                    nc.sync.dma_start(wo_sb[:, :, sl], wo_r[:, :, sl])
                for b in range(B):
                    for gqt in range(S // P):
                        for dc in range(n_dcol):
                            pso = psum_o.tile([P, DCOL], F32, tag="ps_o")
                            for h in range(HPC):
                                nc.tensor.matmul(
                                    pso[:],
                                    _r(ctx_tiles[(b, h)]
                                       [:, gqt * P:(gqt + 1) * P]),
                                    _r(wo_sb[:, h, dc * DCOL:(dc + 1) * DCOL]),
                                    start=(h == 0), stop=(h == HPC - 1))
                            ost = opool.tile([P, DCOL], F32, tag="ost")
                            if (gqt + dc) % 2 == 0:
                                nc.scalar.copy(ost[:], pso[:])
                            else:
                                nc.vector.tensor_copy(ost[:], pso[:])
                            nc.sync.dma_start(
                                out[b * S + gqt * P:b * S + (gqt + 1) * P,
                                    dc * DCOL:(dc + 1) * DCOL], ost[:])

    nc.compile()
    return nc


def causal_nkb_tab(mask, KB):
    """nkb_tab from the actual bool mask [B, S, S] (general, not just tril)."""
    B, S, _ = mask.shape
    tab = []
    for b in range(B):
        row = []
        for qt in range(S // P):
            m = mask[b, qt * P:(qt + 1) * P, :]
            anyk = np.nonzero(m.any(axis=0))[0]
            last = int(anyk[-1]) if len(anyk) else 0
            row.append(last // KB + 1)
        tab.append(row)
    return tab


def shard_inputs(hidden_q, hidden_kv, attention_mask, position_bias,
                 Wq, Wk, Wv, Wo, n_cores=8):
    hidden_q = np.asarray(hidden_q, np.float32)
    hidden_kv = np.asarray(hidden_kv, np.float32)
    attention_mask = np.asarray(attention_mask, bool)
    position_bias = np.asarray(position_bias, np.float32)
    Wq = np.asarray(Wq, np.float32)
    Wk = np.asarray(Wk, np.float32)
    Wv = np.asarray(Wv, np.float32)
    Wo = np.asarray(Wo, np.float32)

    B, S, D = hidden_q.shape
    H = position_bias.shape[1]
    HPC = H // n_cores
    WPC = HPC * DH
    scale = np.float32(1.0 / np.sqrt(DH))

    xq = np.ascontiguousarray(hidden_q.reshape(B * S, D).T)
    xk = np.ascontiguousarray(hidden_kv.reshape(B * S, D).T)
    mask_add = np.where(attention_mask, np.float32(0.0),
                        np.float32(NEG))[:, None]   # [B,1,S,S]
    Wq_s = Wq * scale

    in_maps = []
    for c in range(n_cores):
        sl = slice(c * WPC, (c + 1) * WPC)
        pbm = (position_bias[:, c * HPC:(c + 1) * HPC] + mask_add)
        pbm = pbm.astype(ml_dtypes.bfloat16)
        in_maps.append({
            "xqT": xq,
            "xkT": xk,
            "wq": np.ascontiguousarray(Wq_s[:, sl]),
            "wk": np.ascontiguousarray(Wk[:, sl]),
            "wv": np.ascontiguousarray(Wv[:, sl]),
            "wo": np.ascontiguousarray(Wo[sl, :]),
            "pbm": np.ascontiguousarray(pbm),
        })
    meta = dict(B=B, S=S, D=D, HPC=HPC,
                nkb_tab=causal_nkb_tab(attention_mask, min(512, S)))
    return in_maps, meta


_PROG_CACHE = {}


def _get_program(B, S, D, HPC, nkb_key, n_cores):
    key = (B, S, D, HPC, nkb_key, n_cores)
    if key not in _PROG_CACHE:
        _PROG_CACHE[key] = build_program(
            B, S, D, HPC, [list(r) for r in nkb_key], n_cores)
    return _PROG_CACHE[key]


def kernel(hidden_q, hidden_kv, attention_mask, position_bias,
           Wq, Wk, Wv, Wo):
    n_cores = 8
    in_maps, meta = shard_inputs(hidden_q, hidden_kv, attention_mask,
                                 position_bias, Wq, Wk, Wv, Wo, n_cores)
    nkb_key = tuple(tuple(r) for r in meta["nkb_tab"])
    nc = _get_program(meta["B"], meta["S"], meta["D"], meta["HPC"],
                      nkb_key, n_cores)

    from concourse.bass_utils import run_bass_kernel_spmd
    res = None
    for attempt in range(3):
        try:
            res = run_bass_kernel_spmd(nc, in_maps, list(range(n_cores)))
            break
        except Exception:
            # Transient NRT_EXEC_UNIT_UNRECOVERABLE wedges recover on a
            # fresh PJRT client; reset backends and retry.
            if attempt == 2:
                raise
            try:
                import time as _time

                import jax as _jax
                _jax.clear_caches()
                _jax.extend.backend.clear_backends()
                _time.sleep(15 * (attempt + 1))
            except Exception:
                pass

    B, S, D = meta["B"], meta["S"], meta["D"]
    acc = np.zeros((B * S, D), np.float32)
    for r in res.results:
        acc += r["out"]
    return acc.reshape(B, S, D)



# revision 15
# speedup vs baseline: 1.2744x; 1.2744x over previous
"""CPMAnt attention kernel for Trainium2, 8-core tensor-parallel over heads.

Math (per reference):
    q = hq @ Wq; k = hkv @ Wk; v = hkv @ Wv           (heads split col-wise)
    score = (q k^T) / sqrt(dh) + position_bias, masked
    probs = softmax(score);  out = (probs @ v) @ Wo    (Wo split row-wise)

Sharding: core c owns heads [HPC*c, HPC*(c+1)): Wq/Wk/Wv column slices,
Wo row slice, position_bias head slice. Each core returns a partial
output (its heads' contribution through Wo); the host sums the 8
partials (the "all-reduce" of the row-parallel projection).

v3 design notes (all matmul operands bf16, f32 PSUM accumulate):
 - hidden states ship pre-transposed+bf16 (X^T, [D, B*S]); contraction
   dim D on SBUF partitions for the projections.
 - Q^T/K^T ([dh, seq] per head) and V ([seq, dh+1] per head, ones
   column appended) stay RESIDENT in SBUF — no DRAM scratch.
 - scores computed TRANSPOSED: score^T[k, q] = (K^T blk)^T x (Q^T
   cols); probs^T feeds the PV matmul as lhsT directly — no PE
   transposes of probs.
 - bias+mask: host precomputes ebT[b,h,k,q] = exp(position_bias) *
   mask (bf16, 0 where masked); device does exp(score) on ACT then one
   DVE bf16 multiply: exp(s+b) = exp(s)*exp(b). No PE bias matmuls.
 - softmax sums ride the PV matmul via the V ones column; ctx lands
   [q, dh+1] in PSUM with row sums at col dh, so 1/sum is a plain
   per-partition scalar multiply. No max-subtraction pass.
 - phase C needs ctx^T: one 128x128 bf16 PE transpose per
   (b,h,qtile), emitted AFTER all PV matmuls of the super-tile so the
   DVE normalize latency hides under PV work.
 - causal/general-mask skip at 128 granularity both in score^T (q
   column trim) and PV (kb <= kbcnt-1).
 - single-tag PSUM pool: every PSUM tile is <=1 bank; one bufs=8 ring
   shared by all phases -> no pool-close barriers on PSUM.
 - DMA instruction count minimized (~200/rep): X and W loads batched
   8 d-chunks per DMA, ebT batched per (b,h,sup), output written 1024
   cols per DMA. (Each DMA instruction costs ~625ns of serialized
   HWDGE dispatch.)
 - emission interleaves phases for PE-queue overlap:
   A1, A2(b0), {A2(b1,n) | B(b0,h)}, {B(b1,h) | C(b0)}, C(b1).
 - scale 1/sqrt(dh) folded into Wq on the host.
"""

import contextlib
import sys

sys.path.insert(0, "/opt/trn_rl_repo")

import ml_dtypes
import numpy as np

import concourse.bacc as bacc
import concourse.mybir as mybir
import concourse.tile as tile
from concourse.masks import make_identity

F32 = mybir.dt.float32
BF16 = mybir.dt.bfloat16
AF = mybir.ActivationFunctionType
P = 128
DH = 128


def build_program(B, S, D, HPC, kbcnt, n_cores=8, reps=1):
    """Emit the per-core SPMD program (identical on every core).

    kbcnt[b][gqt]: number of 128-wide key blocks with any unmasked key
    for the 128-row q-tile gqt of batch b.
    reps>1 wraps the body in a For_i loop (benchmarking only).
    """
    WPC = HPC * DH
    BS = B * S
    n_d = D // P
    QSUP = min(512, S)        # q super-tile in phase B
    n_qts = QSUP // P
    n_sup = S // QSUP
    NQT = S // P              # q tiles per batch
    ST1 = 512                 # phase-A1 seq tile
    ST2 = 256                 # phase-A2 seq tile
    DCOL = min(512, D)
    n_dcol = D // DCOL
    DB = 8 if n_d % 8 == 0 else 1    # d-chunks per X/W DMA
    n_wch = n_d // DB
    nA1 = BS // ST1
    nA2 = BS // ST2

    def q_lo(b, sup, kb):
        """First valid q column (within super-tile sup) for key block kb."""
        for qt in range(n_qts):
            if kbcnt[b][sup * n_qts + qt] > kb:
                return qt * P
        return None

    nc = bacc.Bacc("TRN2", target_bir_lowering=False, debug=False,
                   num_devices=n_cores)
    xqT = nc.dram_tensor("xqT", [D, BS], BF16, kind="ExternalInput").ap()
    xkT = nc.dram_tensor("xkT", [D, BS], BF16, kind="ExternalInput").ap()
    wq = nc.dram_tensor("wq", [D, WPC], BF16, kind="ExternalInput").ap()
    wk = nc.dram_tensor("wk", [D, WPC], BF16, kind="ExternalInput").ap()
    wv = nc.dram_tensor("wv", [D, WPC], BF16, kind="ExternalInput").ap()
    wo = nc.dram_tensor("wo", [WPC, D], BF16, kind="ExternalInput").ap()
    ebm = nc.dram_tensor("ebm", [B, HPC, S, S], BF16,
                         kind="ExternalInput").ap()
    out = nc.dram_tensor("out", [BS, D], BF16, kind="ExternalOutput").ap()

    xq_r = xqT.rearrange("(a p) c -> p a c", p=P)
    xk_r = xkT.rearrange("(a p) c -> p a c", p=P)
    wq_r = wq.rearrange("(a p) c -> p a c", p=P)
    wk_r = wk.rearrange("(a p) c -> p a c", p=P)
    wv_r = wv.rearrange("(a p) c -> p a c", p=P)
    wo_r = wo.rearrange("(h p) d -> p h d", p=P)
    eb_r = ebm.rearrange("b h (kb p) q -> p b h kb q", p=P)

    with tile.TileContext(nc, pool_alloc_mode="queue") as tc, \
            contextlib.ExitStack() as es:
        cpool = es.enter_context(tc.tile_pool(name="const", bufs=1))
        res = es.enter_context(tc.tile_pool(name="resident", bufs=1))
        pp = es.enter_context(tc.tile_pool(name="ps", bufs=8, space="PSUM"))

        ident_f = cpool.tile([P, P], F32, tag="ident_f")
        make_identity(nc, ident_f[:])
        ident_bf = cpool.tile([P, P], BF16, tag="ident_bf")
        nc.vector.tensor_copy(ident_bf[:], ident_f[:])

        qt_sb = res.tile([P, HPC, BS], BF16, tag="qt_sb")
        kt_sb = res.tile([P, HPC, BS], BF16, tag="kt_sb")
        v_sb = res.tile([P, BS // P, HPC, DH + 1], BF16, tag="v_sb")
        ctxT_sb = res.tile([P, B, HPC, S], BF16, tag="ctxT_sb")
        # ones column for the PV row-sum trick (phase-A copies only
        # write cols [0, DH), so it survives reps)
        nc.gpsimd.memset(v_sb[:, :, :, DH:DH + 1], 1.0)

        rep_cm = tc.For_i(0, reps, 1) if reps > 1 else contextlib.nullcontext()
        with rep_cm:
            # Weights rotate through a 2-slot pool on the RIGHT SBUF
            # stack (wq -> s0, wk -> s1, wv -> s0 after A1 frees wq).
            # The wv load dispatches from the idle GpSimd queue (SWDGE)
            # so its wait-for-wq-free doesn't head-of-line block the SP
            # DMA stream.
            a2_es = contextlib.ExitStack()
            wp = a2_es.enter_context(
                tc.tile_pool(name="wab", bufs=2, side="right"))
            xp2 = a2_es.enter_context(
                tc.tile_pool(name="xa2", bufs=2, side="right"))
            a1_es = contextlib.ExitStack()
            xp1 = a1_es.enter_context(tc.tile_pool(name="xa1", bufs=3))
            if True:
                # ---------- Phase A1: Q^T = (X Wq)^T, [dh, seq]/head ----
                wq_sb = wp.tile([P, n_d, WPC], BF16, tag="w", name="wq_sb")
                wk_sb = wp.tile([P, n_d, WPC], BF16, tag="w", name="wk_sb")
                nc.sync.dma_start(wq_sb[:, 0:DB, :], wq_r[:, 0:DB, :])
                for n in range(nA1):
                    ps_m = [pp.tile([P, ST1], F32, tag="ps", name=f"psa{m}")
                            for m in range(HPC)]
                    for c in range(n_d // DB):
                        # stage wq chunk c before its first use (chunk c
                        # transfers while chunk c-1's matmuls run)
                        if n == 0 and c > 0:
                            sl = slice(c * DB, (c + 1) * DB)
                            nc.sync.dma_start(wq_sb[:, sl, :], wq_r[:, sl, :])
                        xt = xp1.tile([P, DB, ST1], BF16, tag="xt")
                        nc.sync.dma_start(
                            xt[:], xq_r[:, c * DB:(c + 1) * DB,
                                        n * ST1:(n + 1) * ST1])
                        for j in range(DB):
                            d = c * DB + j
                            for m in range(HPC):
                                nc.tensor.matmul(
                                    ps_m[m][:],
                                    wq_sb[:, d, m * P:(m + 1) * P],
                                    xt[:, j, :],
                                    start=(d == 0), stop=(d == n_d - 1))
                    for m in range(HPC):
                        nc.scalar.copy(
                            qt_sb[:, m, n * ST1:(n + 1) * ST1], ps_m[m][:])
                    # prefetch wk while A1 computes
                    if n < n_wch:
                        sl = slice(n * DB, (n + 1) * DB)
                        nc.sync.dma_start(wk_sb[:, sl, :], wk_r[:, sl, :])
                a1_es.close()  # frees A1 x-tile space for B/C pools
                wv_sb = wp.tile([P, n_d, WPC], BF16, tag="w", name="wv_sb")
                for c in range(2):
                    sl = slice(c * n_d // 2, (c + 1) * n_d // 2)
                    nc.gpsimd.dma_start(wv_sb[:, sl, :], wv_r[:, sl, :])

                # ---------- Phase A2: K^T and V (one pass over X_kv^T) --
                DB2 = n_d // 2
                n_s2 = ST2 // P

                def a2_step(n):
                    xts = []
                    for c in range(2):
                        xt = xp2.tile([P, DB2, ST2], BF16, tag="xt2")
                        nc.sync.dma_start(
                            xt[:], xk_r[:, c * DB2:(c + 1) * DB2,
                                        n * ST2:(n + 1) * ST2])
                        xts.append(xt)
                    psk = [pp.tile([P, ST2], F32, tag="ps",
                                   name=f"psk{m}") for m in range(HPC)]
                    for c in range(2):
                        for j in range(DB2):
                            d = c * DB2 + j
                            for m in range(HPC):
                                nc.tensor.matmul(
                                    psk[m][:],
                                    wk_sb[:, d, m * P:(m + 1) * P],
                                    xts[c][:, j, :],
                                    start=(d == 0), stop=(d == n_d - 1))
                    for m in range(HPC):
                        nc.scalar.copy(
                            kt_sb[:, m, n * ST2:(n + 1) * ST2], psk[m][:])
                    psv = [pp.tile([P, WPC], F32, tag="ps", name=f"psv{s2}")
                           for s2 in range(n_s2)]
                    for c in range(2):
                        for j in range(DB2):
                            d = c * DB2 + j
                            for s2 in range(n_s2):
                                nc.tensor.matmul(
                                    psv[s2][:],
                                    xts[c][:, j, s2 * P:(s2 + 1) * P],
                                    wv_sb[:, d, :],
                                    start=(d == 0), stop=(d == n_d - 1))
                    for s2 in range(n_s2):
                        blk = n * n_s2 + s2
                        for h in range(HPC):
                            nc.vector.tensor_copy(
                                v_sb[:, blk, h, :DH],
                                psv[s2][:, h * P:(h + 1) * P])

                # ---------- Phase B: one (b, h) attention block, split --
                # b_scores emits score^T + exp + eb-multiply; b_pv emits
                # the PV matmuls + normalize + ctx transposes. A big A2/C
                # matmul block goes between them in the PE queue so the
                # ACT/DVE probs latency is fully hidden.
                pending = {}

                def b_scores(b, h):
                    probs = []
                    for sup in range(n_sup):
                        kbmax = max(kbcnt[b][sup * n_qts + qt]
                                    for qt in range(n_qts))
                        eb = ebp.tile([P, 8, QSUP], BF16, tag="eb")
                        nc.sync.dma_start(
                            eb[:, :kbmax, :],
                            eb_r[:, b, h, :kbmax,
                                 sup * QSUP:(sup + 1) * QSUP])
                        probs_tiles = [None] * kbmax
                        for kb in range(kbmax):
                            qlo = q_lo(b, sup, kb)
                            if qlo is None:
                                continue
                            ps_s = pp.tile([P, QSUP], F32, tag="ps",
                                           name="ps_s")
                            q0 = b * S + sup * QSUP
                            nc.tensor.matmul(
                                ps_s[:, qlo:],
                                kt_sb[:, h, b * S + kb * P:
                                      b * S + (kb + 1) * P],
                                qt_sb[:, h, q0 + qlo:q0 + QSUP],
                                start=True, stop=True)
                            pr = prp.tile([P, QSUP], BF16, tag="pr")
                            nc.scalar.activation(
                                pr[:, qlo:], ps_s[:, qlo:], AF.Exp)
                            nc.vector.tensor_mul(
                                pr[:, qlo:], pr[:, qlo:], eb[:, kb, qlo:])
                            probs_tiles[kb] = pr
                        probs.append(probs_tiles)
                    pending[(b, h)] = probs

                def b_pv(b, h):
                    probs = pending.pop((b, h))
                    cxs = []
                    for sup in range(n_sup):
                        for qt in range(n_qts):
                            gqt = sup * n_qts + qt
                            nkb = kbcnt[b][gqt]
                            ps_c = pp.tile([P, DH + 1], F32, tag="ps",
                                           name="ps_c")
                            for kb in range(nkb):
                                nc.tensor.matmul(
                                    ps_c[:],
                                    probs[sup][kb][:, qt * P:(qt + 1) * P],
                                    v_sb[:, b * NQT + kb, h, :],
                                    start=(kb == 0), stop=(kb == nkb - 1))
                            recip = smp.tile([P, 1], F32, tag="recip",
                                             bufs=8)
                            nc.vector.reciprocal(recip[:],
                                                 ps_c[:, DH:DH + 1])
                            cx = smp.tile([P, P], BF16, tag="cx", bufs=6)
                            nc.vector.tensor_scalar_mul(
                                cx[:], ps_c[:, :DH], recip[:])
                            cxs.append(cx)
                    for gqt in range(NQT):
                        ps_t = pp.tile([P, P], BF16, tag="ps", name="ps_t")
                        nc.tensor.transpose(
                            ps_t[:], cxs[gqt][:], ident_bf[:])
                        nc.vector.tensor_copy(
                            ctxT_sb[:, b, h, gqt * P:(gqt + 1) * P],
                            ps_t[:])

                # ---------- Phase C step: 2 Wo col-chunks for batch b ---
                # Wo streams per (b, dc2) chunk (8KB/partition resident
                # instead of 32KB).
                def c_step(b, dc2):
                    woc = wopool.tile([P, HPC, 2 * DCOL], BF16, tag="woc")
                    nc.sync.dma_start(
                        woc[:], wo_r[:, :, dc2 * 2 * DCOL:
                                     (dc2 + 1) * 2 * DCOL])
                    for gqt in range(NQT):
                        ost = opool.tile([P, 2 * DCOL], BF16, tag="ost")
                        for k in range(2):
                            dc = dc2 * 2 + k
                            pso = pp.tile([P, DCOL], F32, tag="ps",
                                          name="ps_o")
                            for h in range(HPC):
                                nc.tensor.matmul(
                                    pso[:],
                                    ctxT_sb[:, b, h, gqt * P:(gqt + 1) * P],
                                    woc[:, h, k * DCOL:(k + 1) * DCOL],
                                    start=(h == 0), stop=(h == HPC - 1))
                            if (gqt + dc) % 2 == 0:
                                nc.scalar.copy(ost[:, k * DCOL:
                                                   (k + 1) * DCOL], pso[:])
                            else:
                                nc.vector.tensor_copy(
                                    ost[:, k * DCOL:(k + 1) * DCOL], pso[:])
                        nc.sync.dma_start(
                            out[b * S + gqt * P:b * S + (gqt + 1) * P,
                                dc2 * 2 * DCOL:(dc2 + 1) * 2 * DCOL],
                            ost[:])

                with (
                    tc.tile_pool(name="beb", bufs=2) as ebp,
                    tc.tile_pool(name="bpr", bufs=14) as prp,
                    tc.tile_pool(name="bsm", bufs=1) as smp,
                    tc.tile_pool(name="cw", bufs=2) as wopool,
                    tc.tile_pool(name="co", bufs=2) as opool,
                ):
                    # A2 over batch 0, then interleave A2(b1) with B(b0),
                    # then B(b1) with C(b0), then C(b1). Each B block's
                    # scores are emitted BEFORE the neighboring A2/C
                    # matmul burst and its PVs AFTER, so the exp/mul
                    # latency hides under ~14-27us of PE work.
                    for n in range(nA2 // B):
                        a2_step(n)
                    assert nA2 // B == HPC  # one A2 step per B head-block
                    for h in range(HPC):
                        b_scores(0, h)
                        a2_step(nA2 // B + h)
                        b_pv(0, h)
                    a2_es.close()  # frees wq/wk/wv + A2 x-tile space
                    assert n_dcol // 2 == HPC  # one C chunk per B block
                    for h in range(HPC):
                        b_scores(1, h)
                        c_step(0, h)
                        b_pv(1, h)
                    for dc2 in range(n_dcol // 2):
                        c_step(1, dc2)

    nc.compile()
    return nc


def mask_kbcnt(mask):
    """kbcnt[b][gqt] from the bool mask [B, S, S] (general, not just
    tril): number of 128-wide key blocks up to the last unmasked key."""
    B, S, _ = mask.shape
    tab = []
    for b in range(B):
        row = []
        for qt in range(S // P):
            m = mask[b, qt * P:(qt + 1) * P, :]
            anyk = np.nonzero(m.any(axis=0))[0]
            last = int(anyk[-1]) if len(anyk) else 0
            row.append(last // P + 1)
        tab.append(row)
    return tab


def shard_inputs(hidden_q, hidden_kv, attention_mask, position_bias,
                 Wq, Wk, Wv, Wo, n_cores=8):
    hidden_q = np.asarray(hidden_q, np.float32)
    hidden_kv = np.asarray(hidden_kv, np.float32)
    attention_mask = np.asarray(attention_mask, bool)
    position_bias = np.asarray(position_bias, np.float32)
    Wq = np.asarray(Wq, np.float32)
    Wk = np.asarray(Wk, np.float32)
    Wv = np.asarray(Wv, np.float32)
    Wo = np.asarray(Wo, np.float32)

    B, S, D = hidden_q.shape
    H = position_bias.shape[1]
    HPC = H // n_cores
    WPC = HPC * DH
    scale = np.float32(1.0 / np.sqrt(DH))

    bf = ml_dtypes.bfloat16
    xq = np.ascontiguousarray(hidden_q.reshape(B * S, D).T).astype(bf)
    xk = np.ascontiguousarray(hidden_kv.reshape(B * S, D).T).astype(bf)
    # ebT[b,h,k,q] = exp(position_bias)*mask, transposed on (q,k)
    ebT = (np.exp(position_bias)
           * attention_mask[:, None, :, :]).transpose(0, 1, 3, 2)
    Wq_s = Wq * scale

    in_maps = []
    for c in range(n_cores):
        sl = slice(c * WPC, (c + 1) * WPC)
        in_maps.append({
            "xqT": xq,
            "xkT": xk,
            "wq": np.ascontiguousarray(Wq_s[:, sl]).astype(bf),
            "wk": np.ascontiguousarray(Wk[:, sl]).astype(bf),
            "wv": np.ascontiguousarray(Wv[:, sl]).astype(bf),
            "wo": np.ascontiguousarray(Wo[sl, :]).astype(bf),
            "ebm": np.ascontiguousarray(
                ebT[:, c * HPC:(c + 1) * HPC]).astype(bf),
        })
    meta = dict(B=B, S=S, D=D, HPC=HPC, nkb_tab=mask_kbcnt(attention_mask))
    return in_maps, meta


_PROG_CACHE = {}


def _get_program(B, S, D, HPC, nkb_key, n_cores):
    key = (B, S, D, HPC, nkb_key, n_cores)
    if key not in _PROG_CACHE:
        _PROG_CACHE[key] = build_program(
            B, S, D, HPC, [list(r) for r in nkb_key], n_cores)
    return _PROG_CACHE[key]


def kernel(hidden_q, hidden_kv, attention_mask, position_bias,
           Wq, Wk, Wv, Wo):
    n_cores = 8
    in_maps, meta = shard_inputs(hidden_q, hidden_kv, attention_mask,
                                 position_bias, Wq, Wk, Wv, Wo, n_cores)
    nkb_key = tuple(tuple(r) for r in meta["nkb_tab"])
    nc = _get_program(meta["B"], meta["S"], meta["D"], meta["HPC"],
                      nkb_key, n_cores)

    from concourse.bass_utils import run_bass_kernel_spmd
    res = None
    for attempt in range(3):
        try:
            res = run_bass_kernel_spmd(nc, in_maps, list(range(n_cores)))
            break
        except Exception:
            # Transient NRT_EXEC_UNIT_UNRECOVERABLE wedges recover on a
            # fresh PJRT client; reset backends and retry.
            if attempt == 2:
                raise
            try:
                import time as _time

                import jax as _jax
                _jax.clear_caches()
                _jax.extend.backend.clear_backends()
                _time.sleep(15 * (attempt + 1))
            except Exception:
                pass

    B, S, D = meta["B"], meta["S"], meta["D"]
    acc = np.zeros((B * S, D), np.float32)
    for r in res.results:
        acc += np.asarray(r["out"], np.float32)
    return acc.reshape(B, S, D)


# revision 21
# speedup vs baseline: 1.2761x; 1.0013x over previous
"""CPMAnt attention kernel for Trainium2, 8-core tensor-parallel over heads.

Math (per reference):
    q = hq @ Wq; k = hkv @ Wk; v = hkv @ Wv           (heads split col-wise)
    score = (q k^T) / sqrt(dh) + position_bias, masked
    probs = softmax(score);  out = (probs @ v) @ Wo    (Wo split row-wise)

Sharding: core c owns heads [HPC*c, HPC*(c+1)): Wq/Wk/Wv column slices,
Wo row slice, position_bias head slice. Each core returns a partial
output (its heads' contribution through Wo); the host sums the 8
partials (the "all-reduce" of the row-parallel projection).

v3 design notes (all matmul operands bf16, f32 PSUM accumulate):
 - hidden states ship pre-transposed+bf16 (X^T, [D, B*S]); contraction
   dim D on SBUF partitions for the projections.
 - Q^T/K^T ([dh, seq] per head) and V ([seq, dh+1] per head, ones
   column appended) stay RESIDENT in SBUF — no DRAM scratch.
 - scores computed TRANSPOSED: score^T[k, q] = (K^T blk)^T x (Q^T
   cols); probs^T feeds the PV matmul as lhsT directly — no PE
   transposes of probs.
 - bias+mask: host precomputes ebT[b,h,k,q] = exp(position_bias) *
   mask (bf16, 0 where masked); device does exp(score) on ACT then one
   DVE bf16 multiply: exp(s+b) = exp(s)*exp(b). No PE bias matmuls.
 - softmax sums ride the PV matmul via the V ones column; ctx lands
   [q, dh+1] in PSUM with row sums at col dh, so 1/sum is a plain
   per-partition scalar multiply. No max-subtraction pass.
 - phase C needs ctx^T: one 128x128 bf16 PE transpose per
   (b,h,qtile), emitted AFTER all PV matmuls of the super-tile so the
   DVE normalize latency hides under PV work.
 - causal/general-mask skip at 128 granularity both in score^T (q
   column trim) and PV (kb <= kbcnt-1).
 - single-tag PSUM pool: every PSUM tile is <=1 bank; one bufs=8 ring
   shared by all phases -> no pool-close barriers on PSUM.
 - DMA instruction count minimized (~200/rep): X and W loads batched
   8 d-chunks per DMA, ebT batched per (b,h,sup), output written 1024
   cols per DMA. (Each DMA instruction costs ~625ns of serialized
   HWDGE dispatch.)
 - emission interleaves phases for PE-queue overlap:
   A1, A2(b0), {A2(b1,n) | B(b0,h)}, {B(b1,h) | C(b0)}, C(b1).
 - scale 1/sqrt(dh) folded into Wq on the host.
"""

import contextlib
import sys

sys.path.insert(0, "/opt/trn_rl_repo")

import ml_dtypes
import numpy as np

import concourse.bacc as bacc
import concourse.mybir as mybir
import concourse.tile as tile
from concourse.masks import make_identity

F32 = mybir.dt.float32
BF16 = mybir.dt.bfloat16
AF = mybir.ActivationFunctionType
P = 128
DH = 128


def build_program(B, S, D, HPC, kbcnt, n_cores=8, reps=1):
    """Emit the per-core SPMD program (identical on every core).

    kbcnt[b][gqt]: number of 128-wide key blocks with any unmasked key
    for the 128-row q-tile gqt of batch b.
    reps>1 wraps the body in a For_i loop (benchmarking only).
    """
    WPC = HPC * DH
    BS = B * S
    n_d = D // P
    QSUP = min(512, S)        # q super-tile in phase B
    n_qts = QSUP // P
    n_sup = S // QSUP
    NQT = S // P              # q tiles per batch
    ST1 = 512                 # phase-A1 seq tile
    ST2 = 256                 # phase-A2 seq tile
    DCOL = min(512, D)
    n_dcol = D // DCOL
    DB = 8 if n_d % 8 == 0 else 1    # d-chunks per X/W DMA
    n_wch = n_d // DB
    nA1 = BS // ST1
    nA2 = BS // ST2

    def q_lo(b, sup, kb):
        """First valid q column (within super-tile sup) for key block kb."""
        for qt in range(n_qts):
            if kbcnt[b][sup * n_qts + qt] > kb:
                return qt * P
        return None

    nc = bacc.Bacc("TRN2", target_bir_lowering=False, debug=False,
                   num_devices=n_cores)
    xqT = nc.dram_tensor("xqT", [D, BS], BF16, kind="ExternalInput").ap()
    xkT = nc.dram_tensor("xkT", [D, BS], BF16, kind="ExternalInput").ap()
    wq = nc.dram_tensor("wq", [D, WPC], BF16, kind="ExternalInput").ap()
    wk = nc.dram_tensor("wk", [D, WPC], BF16, kind="ExternalInput").ap()
    wv = nc.dram_tensor("wv", [D, WPC], BF16, kind="ExternalInput").ap()
    wo = nc.dram_tensor("wo", [WPC, D], BF16, kind="ExternalInput").ap()
    ebm = nc.dram_tensor("ebm", [B, HPC, S, S], BF16,
                         kind="ExternalInput").ap()
    out = nc.dram_tensor("out", [BS, D], BF16, kind="ExternalOutput").ap()

    xq_r = xqT.rearrange("(a p) c -> p a c", p=P)
    xk_r = xkT.rearrange("(a p) c -> p a c", p=P)
    wq_r = wq.rearrange("(a p) c -> p a c", p=P)
    wk_r = wk.rearrange("(a p) c -> p a c", p=P)
    wv_r = wv.rearrange("(a p) c -> p a c", p=P)
    wo_r = wo.rearrange("(h p) d -> p h d", p=P)
    eb_r = ebm.rearrange("b h (kb p) q -> p b h kb q", p=P)

    with tile.TileContext(nc, pool_alloc_mode="queue") as tc, \
            contextlib.ExitStack() as es:
        cpool = es.enter_context(tc.tile_pool(name="const", bufs=1))
        res = es.enter_context(tc.tile_pool(name="resident", bufs=1))
        pp = es.enter_context(tc.tile_pool(name="ps", bufs=8, space="PSUM"))

        ident_f = cpool.tile([P, P], F32, tag="ident_f")
        make_identity(nc, ident_f[:])
        ident_bf = cpool.tile([P, P], BF16, tag="ident_bf")
        nc.vector.tensor_copy(ident_bf[:], ident_f[:])

        qt_sb = res.tile([P, HPC, BS], BF16, tag="qt_sb")
        kt_sb = res.tile([P, HPC, BS], BF16, tag="kt_sb")
        v_sb = res.tile([P, BS // P, HPC, DH + 1], BF16, tag="v_sb")
        ctxT_sb = res.tile([P, B, HPC, S], BF16, tag="ctxT_sb")
        # ones column for the PV row-sum trick (phase-A copies only
        # write cols [0, DH), so it survives reps)
        nc.gpsimd.memset(v_sb[:, :, :, DH:DH + 1], 1.0)

        rep_cm = tc.For_i(0, reps, 1) if reps > 1 else contextlib.nullcontext()
        with rep_cm:
            # Weights rotate through a 2-slot pool on the RIGHT SBUF
            # stack (wq -> s0, wk -> s1, wv -> s0 after A1 frees wq).
            # The wv load dispatches from the idle GpSimd queue (SWDGE)
            # so its wait-for-wq-free doesn't head-of-line block the SP
            # DMA stream.
            a2_es = contextlib.ExitStack()
            wp = a2_es.enter_context(
                tc.tile_pool(name="wab", bufs=2, side="right"))
            xp2 = a2_es.enter_context(
                tc.tile_pool(name="xa2", bufs=2, side="right"))
            a1_es = contextlib.ExitStack()
            xp1 = a1_es.enter_context(tc.tile_pool(name="xa1", bufs=3))
            if True:
                # ---------- Phase A1: Q^T = (X Wq)^T, [dh, seq]/head ----
                wq_sb = wp.tile([P, n_d, WPC], BF16, tag="w", name="wq_sb")
                wk_sb = wp.tile([P, n_d, WPC], BF16, tag="w", name="wk_sb")
                # split the first chunk so the d=0/1 matmuls start after
                # a 256KB transfer instead of 1MB
                nc.sync.dma_start(wq_sb[:, 0:2, :], wq_r[:, 0:2, :])
                nc.sync.dma_start(wq_sb[:, 2:DB, :], wq_r[:, 2:DB, :])
                for n in range(nA1):
                    ps_m = [pp.tile([P, ST1], F32, tag="ps", name=f"psa{m}")
                            for m in range(HPC)]
                    for c in range(n_d // DB):
                        # stage wq chunk c before its first use (chunk c
                        # transfers while chunk c-1's matmuls run)
                        if n == 0 and c > 0:
                            sl = slice(c * DB, (c + 1) * DB)
                            nc.sync.dma_start(wq_sb[:, sl, :], wq_r[:, sl, :])
                        xt = xp1.tile([P, DB, ST1], BF16, tag="xt")
                        if n == 0 and c == 0:
                            nc.sync.dma_start(xt[:, 0:2, :],
                                              xq_r[:, 0:2, 0:ST1])
                            nc.sync.dma_start(xt[:, 2:DB, :],
                                              xq_r[:, 2:DB, 0:ST1])
                        else:
                            nc.sync.dma_start(
                                xt[:], xq_r[:, c * DB:(c + 1) * DB,
                                            n * ST1:(n + 1) * ST1])
                        for j in range(DB):
                            d = c * DB + j
                            for m in range(HPC):
                                nc.tensor.matmul(
                                    ps_m[m][:],
                                    wq_sb[:, d, m * P:(m + 1) * P],
                                    xt[:, j, :],
                                    start=(d == 0), stop=(d == n_d - 1))
                    for m in range(HPC):
                        nc.scalar.copy(
                            qt_sb[:, m, n * ST1:(n + 1) * ST1], ps_m[m][:])
                    # prefetch wk while A1 computes
                    if n < n_wch:
                        sl = slice(n * DB, (n + 1) * DB)
                        nc.sync.dma_start(wk_sb[:, sl, :], wk_r[:, sl, :])
                a1_es.close()  # frees A1 x-tile space for B/C pools
                wv_sb = wp.tile([P, n_d, WPC], BF16, tag="w", name="wv_sb")
                for c in range(2):
                    sl = slice(c * n_d // 2, (c + 1) * n_d // 2)
                    nc.gpsimd.dma_start(wv_sb[:, sl, :], wv_r[:, sl, :])

                # ---------- Phase A2: K^T and V (one pass over X_kv^T) --
                DB2 = n_d // 2
                n_s2 = ST2 // P

                def a2_step(n):
                    xts = []
                    for c in range(2):
                        xt = xp2.tile([P, DB2, ST2], BF16, tag="xt2")
                        nc.sync.dma_start(
                            xt[:], xk_r[:, c * DB2:(c + 1) * DB2,
                                        n * ST2:(n + 1) * ST2])
                        xts.append(xt)
                    psk = [pp.tile([P, ST2], F32, tag="ps",
                                   name=f"psk{m}") for m in range(HPC)]
                    for c in range(2):
                        for j in range(DB2):
                            d = c * DB2 + j
                            for m in range(HPC):
                                nc.tensor.matmul(
                                    psk[m][:],
                                    wk_sb[:, d, m * P:(m + 1) * P],
                                    xts[c][:, j, :],
                                    start=(d == 0), stop=(d == n_d - 1))
                    for m in range(HPC):
                        nc.scalar.copy(
                            kt_sb[:, m, n * ST2:(n + 1) * ST2], psk[m][:])
                    psv = [pp.tile([P, WPC], F32, tag="ps", name=f"psv{s2}")
                           for s2 in range(n_s2)]
                    for c in range(2):
                        for j in range(DB2):
                            d = c * DB2 + j
                            for s2 in range(n_s2):
                                nc.tensor.matmul(
                                    psv[s2][:],
                                    xts[c][:, j, s2 * P:(s2 + 1) * P],
                                    wv_sb[:, d, :],
                                    start=(d == 0), stop=(d == n_d - 1))
                    for s2 in range(n_s2):
                        blk = n * n_s2 + s2
                        for h in range(HPC):
                            nc.vector.tensor_copy(
                                v_sb[:, blk, h, :DH],
                                psv[s2][:, h * P:(h + 1) * P])

                # ---------- Phase B: one (b, h) attention block, split --
                # b_scores emits score^T + exp + eb-multiply; b_pv emits
                # the PV matmuls + normalize + ctx transposes. A big A2/C
                # matmul block goes between them in the PE queue so the
                # ACT/DVE probs latency is fully hidden.
                pending = {}

                def b_scores(b, h):
                    probs = []
                    for sup in range(n_sup):
                        kbmax = max(kbcnt[b][sup * n_qts + qt]
                                    for qt in range(n_qts))
                        eb = ebp.tile([P, 8, QSUP], BF16, tag="eb")
                        nc.sync.dma_start(
                            eb[:, :kbmax, :],
                            eb_r[:, b, h, :kbmax,
                                 sup * QSUP:(sup + 1) * QSUP])
                        probs_tiles = [None] * kbmax
                        for kb in range(kbmax):
                            qlo = q_lo(b, sup, kb)
                            if qlo is None:
                                continue
                            ps_s = pp.tile([P, QSUP], F32, tag="ps",
                                           name="ps_s")
                            q0 = b * S + sup * QSUP
                            nc.tensor.matmul(
                                ps_s[:, qlo:],
                                kt_sb[:, h, b * S + kb * P:
                                      b * S + (kb + 1) * P],
                                qt_sb[:, h, q0 + qlo:q0 + QSUP],
                                start=True, stop=True)
                            pr = prp.tile([P, QSUP], BF16, tag="pr")
                            nc.scalar.activation(
                                pr[:, qlo:], ps_s[:, qlo:], AF.Exp)
                            nc.vector.tensor_mul(
                                pr[:, qlo:], pr[:, qlo:], eb[:, kb, qlo:])
                            probs_tiles[kb] = pr
                        probs.append(probs_tiles)
                    pending[(b, h)] = probs

                def b_pv(b, h):
                    probs = pending.pop((b, h))
                    cxs = {}
                    order = list(range(NQT))
                    for gqt in order:
                        sup, qt = divmod(gqt, n_qts)
                        nkb = kbcnt[b][gqt]
                        ps_c = pp.tile([P, DH + 1], F32, tag="ps",
                                       name="ps_c")
                        for kb in range(nkb):
                            nc.tensor.matmul(
                                ps_c[:],
                                probs[sup][kb][:, qt * P:(qt + 1) * P],
                                v_sb[:, b * NQT + kb, h, :],
                                start=(kb == 0), stop=(kb == nkb - 1))
                        recip = smp.tile([P, 1], F32, tag="recip",
                                         bufs=8)
                        nc.vector.reciprocal(recip[:],
                                             ps_c[:, DH:DH + 1])
                        cx = smp.tile([P, P], BF16, tag="cx", bufs=6)
                        nc.vector.tensor_scalar_mul(
                            cx[:], ps_c[:, :DH], recip[:])
                        cxs[gqt] = cx
                    for gqt in order:
                        ps_t = pp.tile([P, P], BF16, tag="ps", name="ps_t")
                        nc.tensor.transpose(
                            ps_t[:], cxs[gqt][:], ident_bf[:])
                        nc.vector.tensor_copy(
                            ctxT_sb[:, b, h, gqt * P:(gqt + 1) * P],
                            ps_t[:])

                # ---------- Phase C step: 2 Wo col-chunks for batch b ---
                # Wo streams per (b, dc2) chunk (8KB/partition resident
                # instead of 32KB); chunks are staged one step ahead so
                # the transfer hides under the previous step's matmuls.
                woc_q = []

                def stage_woc(dc2):
                    woc = wopool.tile([P, HPC, 2 * DCOL], BF16, tag="woc")
                    nc.sync.dma_start(
                        woc[:], wo_r[:, :, dc2 * 2 * DCOL:
                                     (dc2 + 1) * 2 * DCOL])
                    woc_q.append(woc)

                def c_step(b, dc2, stage_next=None):
                    woc = woc_q.pop(0)
                    if stage_next is not None:
                        stage_woc(stage_next)
                    for gqt in range(NQT):
                        ost = opool.tile([P, 2 * DCOL], BF16, tag="ost")
                        for k in range(2):
                            dc = dc2 * 2 + k
                            pso = pp.tile([P, DCOL], F32, tag="ps",
                                          name="ps_o")
                            for h in range(HPC):
                                nc.tensor.matmul(
                                    pso[:],
                                    ctxT_sb[:, b, h, gqt * P:(gqt + 1) * P],
                                    woc[:, h, k * DCOL:(k + 1) * DCOL],
                                    start=(h == 0), stop=(h == HPC - 1))
                            if (gqt + dc) % 2 == 0:
                                nc.scalar.copy(ost[:, k * DCOL:
                                                   (k + 1) * DCOL], pso[:])
                            else:
                                nc.vector.tensor_copy(
                                    ost[:, k * DCOL:(k + 1) * DCOL], pso[:])
                        nc.sync.dma_start(
                            out[b * S + gqt * P:b * S + (gqt + 1) * P,
                                dc2 * 2 * DCOL:(dc2 + 1) * 2 * DCOL],
                            ost[:])

                with (
                    tc.tile_pool(name="beb", bufs=2) as ebp,
                    tc.tile_pool(name="bpr", bufs=14) as prp,
                    tc.tile_pool(name="bsm", bufs=1) as smp,
                    tc.tile_pool(name="cw", bufs=2) as wopool,
                    tc.tile_pool(name="co", bufs=2) as opool,
                ):
                    # A2 over batch 0, then interleave A2(b1) with B(b0),
                    # then B(b1) with C(b0), then C(b1). Each B block's
                    # scores are emitted BEFORE the neighboring A2/C
                    # matmul burst and its PVs AFTER, so the exp/mul
                    # latency hides under ~14-27us of PE work.
                    for n in range(nA2 // B):
                        a2_step(n)
                    assert nA2 // B == HPC  # one A2 step per B head-block
                    for h in range(HPC):
                        b_scores(0, h)
                        a2_step(nA2 // B + h)
                        b_pv(0, h)
                    a2_es.close()  # frees wq/wk/wv + A2 x-tile space
                    assert n_dcol // 2 == HPC  # one C chunk per B block
                    stage_woc(0)
                    for h in range(HPC):
                        b_scores(1, h)
                        c_step(0, h, stage_next=(h + 1) % (n_dcol // 2))
                        b_pv(1, h)
                    for dc2 in range(n_dcol // 2):
                        c_step(1, dc2,
                               stage_next=(dc2 + 1 if dc2 + 1 < n_dcol // 2
                                           else None))

    nc.compile()
    return nc


def mask_kbcnt(mask):
    """kbcnt[b][gqt] from the bool mask [B, S, S] (general, not just
    tril): number of 128-wide key blocks up to the last unmasked key."""
    B, S, _ = mask.shape
    tab = []
    for b in range(B):
        row = []
        for qt in range(S // P):
            m = mask[b, qt * P:(qt + 1) * P, :]
            anyk = np.nonzero(m.any(axis=0))[0]
            last = int(anyk[-1]) if len(anyk) else 0
            row.append(last // P + 1)
        tab.append(row)
    return tab


def shard_inputs(hidden_q, hidden_kv, attention_mask, position_bias,
                 Wq, Wk, Wv, Wo, n_cores=8):
    hidden_q = np.asarray(hidden_q, np.float32)
    hidden_kv = np.asarray(hidden_kv, np.float32)
    attention_mask = np.asarray(attention_mask, bool)
    position_bias = np.asarray(position_bias, np.float32)
    Wq = np.asarray(Wq, np.float32)
    Wk = np.asarray(Wk, np.float32)
    Wv = np.asarray(Wv, np.float32)
    Wo = np.asarray(Wo, np.float32)

    B, S, D = hidden_q.shape
    H = position_bias.shape[1]
    HPC = H // n_cores
    WPC = HPC * DH
    scale = np.float32(1.0 / np.sqrt(DH))

    bf = ml_dtypes.bfloat16
    xq = np.ascontiguousarray(hidden_q.reshape(B * S, D).T).astype(bf)
    xk = np.ascontiguousarray(hidden_kv.reshape(B * S, D).T).astype(bf)
    # ebT[b,h,k,q] = exp(position_bias)*mask, transposed on (q,k)
    ebT = (np.exp(position_bias)
           * attention_mask[:, None, :, :]).transpose(0, 1, 3, 2)
    Wq_s = Wq * scale

    in_maps = []
    for c in range(n_cores):
        sl = slice(c * WPC, (c + 1) * WPC)
        in_maps.append({
            "xqT": xq,
            "xkT": xk,
            "wq": np.ascontiguousarray(Wq_s[:, sl]).astype(bf),
            "wk": np.ascontiguousarray(Wk[:, sl]).astype(bf),
            "wv": np.ascontiguousarray(Wv[:, sl]).astype(bf),
            "wo": np.ascontiguousarray(Wo[sl, :]).astype(bf),
            "ebm": np.ascontiguousarray(
                ebT[:, c * HPC:(c + 1) * HPC]).astype(bf),
        })
    meta = dict(B=B, S=S, D=D, HPC=HPC, nkb_tab=mask_kbcnt(attention_mask))
    return in_maps, meta


_PROG_CACHE = {}


def _get_program(B, S, D, HPC, nkb_key, n_cores):
    key = (B, S, D, HPC, nkb_key, n_cores)
    if key not in _PROG_CACHE:
        _PROG_CACHE[key] = build_program(
            B, S, D, HPC, [list(r) for r in nkb_key], n_cores)
    return _PROG_CACHE[key]


def kernel(hidden_q, hidden_kv, attention_mask, position_bias,
           Wq, Wk, Wv, Wo):
    n_cores = 8
    in_maps, meta = shard_inputs(hidden_q, hidden_kv, attention_mask,
                                 position_bias, Wq, Wk, Wv, Wo, n_cores)
    nkb_key = tuple(tuple(r) for r in meta["nkb_tab"])
    nc = _get_program(meta["B"], meta["S"], meta["D"], meta["HPC"],
                      nkb_key, n_cores)

    from concourse.bass_utils import run_bass_kernel_spmd
    res = None
    for attempt in range(3):
        try:
            res = run_bass_kernel_spmd(nc, in_maps, list(range(n_cores)))
            break
        except Exception:
            # Transient NRT_EXEC_UNIT_UNRECOVERABLE wedges recover on a
            # fresh PJRT client; reset backends and retry.
            if attempt == 2:
                raise
            try:
                import time as _time

                import jax as _jax
                _jax.clear_caches()
                _jax.extend.backend.clear_backends()
                _time.sleep(15 * (attempt + 1))
            except Exception:
                pass

    B, S, D = meta["B"], meta["S"], meta["D"]
    acc = np.zeros((B * S, D), np.float32)
    for r in res.results:
        acc += np.asarray(r["out"], np.float32)
    return acc.reshape(B, S, D)


# revision 25
# speedup vs baseline: 1.2989x; 1.0179x over previous
"""CPMAnt attention kernel for Trainium2, 8-core tensor-parallel over heads.

Math (per reference):
    q = hq @ Wq; k = hkv @ Wk; v = hkv @ Wv           (heads split col-wise)
    score = (q k^T) / sqrt(dh) + position_bias, masked
    probs = softmax(score);  out = (probs @ v) @ Wo    (Wo split row-wise)

Sharding: core c owns heads [HPC*c, HPC*(c+1)): Wq/Wk/Wv column slices,
Wo row slice, position_bias head slice. Each core returns a partial
output (its heads' contribution through Wo); the host sums the 8
partials (the "all-reduce" of the row-parallel projection).

v3 design notes (all matmul operands bf16, f32 PSUM accumulate):
 - hidden states ship pre-transposed+bf16 (X^T, [D, B*S]); contraction
   dim D on SBUF partitions for the projections.
 - Q^T/K^T ([dh, seq] per head) and V ([seq, dh+1] per head, ones
   column appended) stay RESIDENT in SBUF — no DRAM scratch.
 - scores computed TRANSPOSED: score^T[k, q] = (K^T blk)^T x (Q^T
   cols); probs^T feeds the PV matmul as lhsT directly — no PE
   transposes of probs.
 - bias+mask: host precomputes ebT[b,h,k,q] = exp(position_bias) *
   mask (bf16, 0 where masked); device does exp(score) on ACT then one
   DVE bf16 multiply: exp(s+b) = exp(s)*exp(b). No PE bias matmuls.
 - softmax sums ride the PV matmul via the V ones column; ctx lands
   [q, dh+1] in PSUM with row sums at col dh, so 1/sum is a plain
   per-partition scalar multiply. No max-subtraction pass.
 - phase C needs ctx^T: one 128x128 bf16 PE transpose per
   (b,h,qtile), emitted AFTER all PV matmuls of the super-tile so the
   DVE normalize latency hides under PV work.
 - causal/general-mask skip at 128 granularity both in score^T (q
   column trim) and PV (kb <= kbcnt-1).
 - single-tag PSUM pool: every PSUM tile is <=1 bank; one bufs=8 ring
   shared by all phases -> no pool-close barriers on PSUM.
 - DMA instruction count minimized (~200/rep): X and W loads batched
   8 d-chunks per DMA, ebT batched per (b,h,sup), output written 1024
   cols per DMA. (Each DMA instruction costs ~625ns of serialized
   HWDGE dispatch.)
 - emission interleaves phases for PE-queue overlap:
   A1, A2(b0), {A2(b1,n) | B(b0,h)}, {B(b1,h) | C(b0)}, C(b1).
 - scale 1/sqrt(dh) folded into Wq on the host.
"""

import contextlib
import sys

sys.path.insert(0, "/opt/trn_rl_repo")

import ml_dtypes
import numpy as np

import concourse.bacc as bacc
import concourse.mybir as mybir
import concourse.tile as tile
from concourse.masks import make_identity

F32 = mybir.dt.float32
BF16 = mybir.dt.bfloat16
AF = mybir.ActivationFunctionType
P = 128
DH = 128


def build_program(B, S, D, HPC, kbcnt, n_cores=8, reps=1):
    """Emit the per-core SPMD program (identical on every core).

    kbcnt[b][gqt]: number of 128-wide key blocks with any unmasked key
    for the 128-row q-tile gqt of batch b.
    reps>1 wraps the body in a For_i loop (benchmarking only).
    """
    WPC = HPC * DH
    BS = B * S
    n_d = D // P
    QSUP = min(512, S)        # q super-tile in phase B
    n_qts = QSUP // P
    n_sup = S // QSUP
    NQT = S // P              # q tiles per batch
    ST1 = 512                 # phase-A1 seq tile
    ST2 = 256                 # phase-A2 seq tile
    DCOL = min(512, D)
    n_dcol = D // DCOL
    DB = 8 if n_d % 8 == 0 else 1    # d-chunks per X/W DMA
    n_wch = n_d // DB
    nA1 = BS // ST1
    nA2 = BS // ST2

    def q_lo(b, sup, kb):
        """First valid q column (within super-tile sup) for key block kb."""
        for qt in range(n_qts):
            if kbcnt[b][sup * n_qts + qt] > kb:
                return qt * P
        return None

    nc = bacc.Bacc("TRN2", target_bir_lowering=False, debug=False,
                   num_devices=n_cores)
    xqT = nc.dram_tensor("xqT", [D, BS], BF16, kind="ExternalInput").ap()
    xkT = nc.dram_tensor("xkT", [D, BS], BF16, kind="ExternalInput").ap()
    wq = nc.dram_tensor("wq", [D, WPC], BF16, kind="ExternalInput").ap()
    wk = nc.dram_tensor("wk", [D, WPC], BF16, kind="ExternalInput").ap()
    wv = nc.dram_tensor("wv", [D, WPC], BF16, kind="ExternalInput").ap()
    wo = nc.dram_tensor("wo", [WPC, D], BF16, kind="ExternalInput").ap()
    ebm = nc.dram_tensor("ebm", [B, HPC, S, S], BF16,
                         kind="ExternalInput").ap()
    out = nc.dram_tensor("out", [BS, D], BF16, kind="ExternalOutput").ap()

    xq_r = xqT.rearrange("(a p) c -> p a c", p=P)
    xk_r = xkT.rearrange("(a p) c -> p a c", p=P)
    wq_r = wq.rearrange("(a p) c -> p a c", p=P)
    wk_r = wk.rearrange("(a p) c -> p a c", p=P)
    wv_r = wv.rearrange("(a p) c -> p a c", p=P)
    wo_r = wo.rearrange("(h p) d -> p h d", p=P)
    eb_r = ebm.rearrange("b h (kb p) q -> p b h kb q", p=P)

    with tile.TileContext(nc, pool_alloc_mode="queue") as tc, \
            contextlib.ExitStack() as es:
        cpool = es.enter_context(tc.tile_pool(name="const", bufs=1))
        res = es.enter_context(tc.tile_pool(name="resident", bufs=1))
        pp = es.enter_context(tc.tile_pool(name="ps", bufs=8, space="PSUM"))

        ident_f = cpool.tile([P, P], F32, tag="ident_f")
        make_identity(nc, ident_f[:])
        ident_bf = cpool.tile([P, P], BF16, tag="ident_bf")
        nc.vector.tensor_copy(ident_bf[:], ident_f[:])

        qt_sb = res.tile([P, HPC, BS], BF16, tag="qt_sb")
        kt_sb = res.tile([P, HPC, BS], BF16, tag="kt_sb")
        v_sb = res.tile([P, BS // P, HPC, DH + 1], BF16, tag="v_sb")
        ctxT_sb = res.tile([P, B, HPC, S], BF16, tag="ctxT_sb")
        # ones column for the PV row-sum trick (phase-A copies only
        # write cols [0, DH), so it survives reps)
        nc.gpsimd.memset(v_sb[:, :, :, DH:DH + 1], 1.0)

        rep_cm = tc.For_i(0, reps, 1) if reps > 1 else contextlib.nullcontext()
        with rep_cm:
            # Weights rotate through a 2-slot pool on the RIGHT SBUF
            # stack (wq -> s0, wk -> s1, wv -> s0 after A1 frees wq).
            # The wv load dispatches from the idle GpSimd queue (SWDGE)
            # so its wait-for-wq-free doesn't head-of-line block the SP
            # DMA stream.
            a2_es = contextlib.ExitStack()
            wp = a2_es.enter_context(
                tc.tile_pool(name="wab", bufs=2, side="right"))
            xp2 = a2_es.enter_context(
                tc.tile_pool(name="xa2", bufs=2, side="right"))
            a1_es = contextlib.ExitStack()
            xp1 = a1_es.enter_context(tc.tile_pool(name="xa1", bufs=3))
            if True:
                # ---------- Phase A1: Q^T = (X Wq)^T, [dh, seq]/head ----
                wq_sb = wp.tile([P, n_d, WPC], BF16, tag="w", name="wq_sb")
                wk_sb = wp.tile([P, n_d, WPC], BF16, tag="w", name="wk_sb")
                # split the first chunk so the d=0/1 matmuls start after
                # a 256KB transfer instead of 1MB
                nc.sync.dma_start(wq_sb[:, 0:2, :], wq_r[:, 0:2, :])
                nc.sync.dma_start(wq_sb[:, 2:DB, :], wq_r[:, 2:DB, :])
                for n in range(nA1):
                    ps_m = [pp.tile([P, ST1], F32, tag="ps", name=f"psa{m}")
                            for m in range(HPC)]
                    for c in range(n_d // DB):
                        # stage wq chunk c before its first use (chunk c
                        # transfers while chunk c-1's matmuls run)
                        if n == 0 and c > 0:
                            sl = slice(c * DB, (c + 1) * DB)
                            nc.sync.dma_start(wq_sb[:, sl, :], wq_r[:, sl, :])
                        xt = xp1.tile([P, DB, ST1], BF16, tag="xt")
                        if n == 0 and c == 0:
                            nc.sync.dma_start(xt[:, 0:2, :],
                                              xq_r[:, 0:2, 0:ST1])
                            nc.sync.dma_start(xt[:, 2:DB, :],
                                              xq_r[:, 2:DB, 0:ST1])
                        else:
                            nc.sync.dma_start(
                                xt[:], xq_r[:, c * DB:(c + 1) * DB,
                                            n * ST1:(n + 1) * ST1])
                        for j in range(DB):
                            d = c * DB + j
                            for m in range(HPC):
                                nc.tensor.matmul(
                                    ps_m[m][:],
                                    wq_sb[:, d, m * P:(m + 1) * P],
                                    xt[:, j, :],
                                    start=(d == 0), stop=(d == n_d - 1))
                    for m in range(HPC):
                        nc.scalar.copy(
                            qt_sb[:, m, n * ST1:(n + 1) * ST1], ps_m[m][:])
                    # prefetch wk while A1 computes
                    if n < n_wch:
                        sl = slice(n * DB, (n + 1) * DB)
                        nc.sync.dma_start(wk_sb[:, sl, :], wk_r[:, sl, :])
                a1_es.close()  # frees A1 x-tile space for B/C pools
                wv_sb = wp.tile([P, n_d, WPC], BF16, tag="w", name="wv_sb")
                for c in range(2):
                    sl = slice(c * n_d // 2, (c + 1) * n_d // 2)
                    nc.gpsimd.dma_start(wv_sb[:, sl, :], wv_r[:, sl, :])

                # ---------- Phase A2: K^T and V (one pass over X_kv^T) --
                DB2 = n_d // 2
                n_s2 = ST2 // P

                def a2_step(n, v_first=False):
                    xts = []
                    for c in range(2):
                        xt = xp2.tile([P, DB2, ST2], BF16, tag="xt2")
                        nc.sync.dma_start(
                            xt[:], xk_r[:, c * DB2:(c + 1) * DB2,
                                        n * ST2:(n + 1) * ST2])
                        xts.append(xt)
                    def emit_v():
                        psv = [pp.tile([P, WPC], F32, tag="ps",
                                       name=f"psv{s2}")
                               for s2 in range(n_s2)]
                        for c in range(2):
                            for j in range(DB2):
                                d = c * DB2 + j
                                for s2 in range(n_s2):
                                    nc.tensor.matmul(
                                        psv[s2][:],
                                        xts[c][:, j, s2 * P:(s2 + 1) * P],
                                        wv_sb[:, d, :],
                                        start=(d == 0), stop=(d == n_d - 1))
                        for s2 in range(n_s2):
                            blk = n * n_s2 + s2
                            for h in range(HPC):
                                nc.vector.tensor_copy(
                                    v_sb[:, blk, h, :DH],
                                    psv[s2][:, h * P:(h + 1) * P])

                    def emit_k():
                        psk = [pp.tile([P, ST2], F32, tag="ps",
                                       name=f"psk{m}") for m in range(HPC)]
                        for c in range(2):
                            for j in range(DB2):
                                d = c * DB2 + j
                                for m in range(HPC):
                                    nc.tensor.matmul(
                                        psk[m][:],
                                        wk_sb[:, d, m * P:(m + 1) * P],
                                        xts[c][:, j, :],
                                        start=(d == 0), stop=(d == n_d - 1))
                        for m in range(HPC):
                            nc.scalar.copy(
                                kt_sb[:, m, n * ST2:(n + 1) * ST2],
                                psk[m][:])

                    # V-first when a b_pv follows (its psum drains are then
                    # K's fast ACT copies); K-first otherwise (n=0 must not
                    # wait on the SWDGE wv load).
                    if v_first:
                        emit_v()
                        emit_k()
                    else:
                        emit_k()
                        emit_v()

                # ---------- Phase B: one (b, h) attention block, split --
                # b_scores emits score^T + exp + eb-multiply; b_pv emits
                # the PV matmuls + normalize + ctx transposes. A big A2/C
                # matmul block goes between them in the PE queue so the
                # ACT/DVE probs latency is fully hidden.
                pending = {}

                def b_scores(b, h):
                    probs = []
                    for sup in range(n_sup):
                        kbmax = max(kbcnt[b][sup * n_qts + qt]
                                    for qt in range(n_qts))
                        eb = ebp.tile([P, 8, QSUP], BF16, tag="eb")
                        nc.sync.dma_start(
                            eb[:, :kbmax, :],
                            eb_r[:, b, h, :kbmax,
                                 sup * QSUP:(sup + 1) * QSUP])
                        probs_tiles = [None] * kbmax
                        for kb in range(kbmax):
                            qlo = q_lo(b, sup, kb)
                            if qlo is None:
                                continue
                            ps_s = pp.tile([P, QSUP], F32, tag="ps",
                                           name="ps_s")
                            q0 = b * S + sup * QSUP
                            nc.tensor.matmul(
                                ps_s[:, qlo:],
                                kt_sb[:, h, b * S + kb * P:
                                      b * S + (kb + 1) * P],
                                qt_sb[:, h, q0 + qlo:q0 + QSUP],
                                start=True, stop=True)
                            pr = prp.tile([P, QSUP], BF16, tag="pr")
                            nc.scalar.activation(
                                pr[:, qlo:], ps_s[:, qlo:], AF.Exp)
                            nc.vector.tensor_mul(
                                pr[:, qlo:], pr[:, qlo:], eb[:, kb, qlo:])
                            probs_tiles[kb] = pr
                        probs.append(probs_tiles)
                    pending[(b, h)] = probs

                def b_pv(b, h):
                    probs = pending.pop((b, h))
                    cxs = {}
                    order = list(range(NQT))
                    for gqt in order:
                        sup, qt = divmod(gqt, n_qts)
                        nkb = kbcnt[b][gqt]
                        ps_c = pp.tile([P, DH + 1], F32, tag="ps",
                                       name="ps_c")
                        for kb in range(nkb):
                            nc.tensor.matmul(
                                ps_c[:],
                                probs[sup][kb][:, qt * P:(qt + 1) * P],
                                v_sb[:, b * NQT + kb, h, :],
                                start=(kb == 0), stop=(kb == nkb - 1))
                        recip = smp.tile([P, 1], F32, tag="recip",
                                         bufs=8)
                        nc.vector.reciprocal(recip[:],
                                             ps_c[:, DH:DH + 1])
                        cx = smp.tile([P, P], BF16, tag="cx", bufs=6)
                        nc.vector.tensor_scalar_mul(
                            cx[:], ps_c[:, :DH], recip[:])
                        cxs[gqt] = cx
                    for gqt in order:
                        ps_t = pp.tile([P, P], BF16, tag="ps", name="ps_t")
                        nc.tensor.transpose(
                            ps_t[:], cxs[gqt][:], ident_bf[:])
                        nc.vector.tensor_copy(
                            ctxT_sb[:, b, h, gqt * P:(gqt + 1) * P],
                            ps_t[:])

                # ---------- Phase C step: 2 Wo col-chunks for batch b ---
                # Wo streams per (b, dc2) chunk (8KB/partition resident
                # instead of 32KB); chunks are staged one step ahead so
                # the transfer hides under the previous step's matmuls.
                woc_q = []

                def stage_woc(dc2):
                    woc = wopool.tile([P, HPC, 2 * DCOL], BF16, tag="woc")
                    nc.sync.dma_start(
                        woc[:], wo_r[:, :, dc2 * 2 * DCOL:
                                     (dc2 + 1) * 2 * DCOL])
                    woc_q.append(woc)

                def c_step(b, dc2, stage_next=None):
                    woc = woc_q.pop(0)
                    if stage_next is not None:
                        stage_woc(stage_next)
                    for gqt in range(NQT):
                        ost = opool.tile([P, 2 * DCOL], BF16, tag="ost")
                        for k in range(2):
                            dc = dc2 * 2 + k
                            pso = pp.tile([P, DCOL], F32, tag="ps",
                                          name="ps_o")
                            for h in range(HPC):
                                nc.tensor.matmul(
                                    pso[:],
                                    ctxT_sb[:, b, h, gqt * P:(gqt + 1) * P],
                                    woc[:, h, k * DCOL:(k + 1) * DCOL],
                                    start=(h == 0), stop=(h == HPC - 1))
                            if (gqt + dc) % 2 == 0:
                                nc.scalar.copy(ost[:, k * DCOL:
                                                   (k + 1) * DCOL], pso[:])
                            else:
                                nc.vector.tensor_copy(
                                    ost[:, k * DCOL:(k + 1) * DCOL], pso[:])
                        nc.sync.dma_start(
                            out[b * S + gqt * P:b * S + (gqt + 1) * P,
                                dc2 * 2 * DCOL:(dc2 + 1) * 2 * DCOL],
                            ost[:])

                with (
                    tc.tile_pool(name="beb", bufs=2) as ebp,
                    tc.tile_pool(name="bpr", bufs=14) as prp,
                    tc.tile_pool(name="bsm", bufs=1) as smp,
                    tc.tile_pool(name="cw", bufs=2) as wopool,
                    tc.tile_pool(name="co", bufs=2) as opool,
                ):
                    # A2 over batch 0, then interleave A2(b1) with B(b0),
                    # then B(b1) with C(b0), then C(b1). Each B block's
                    # scores are emitted BEFORE the neighboring A2/C
                    # matmul burst and its PVs AFTER, so the exp/mul
                    # latency hides under ~14-27us of PE work.
                    for n in range(nA2 // B):
                        a2_step(n)
                    assert nA2 // B == HPC  # one A2 step per B head-block
                    for h in range(HPC):
                        b_scores(0, h)
                        a2_step(nA2 // B + h, v_first=True)
                        b_pv(0, h)
                    a2_es.close()  # frees wq/wk/wv + A2 x-tile space
                    assert n_dcol // 2 == HPC  # one C chunk per B block
                    stage_woc(0)
                    for h in range(HPC):
                        b_scores(1, h)
                        c_step(0, h, stage_next=(h + 1) % (n_dcol // 2))
                        b_pv(1, h)
                    for dc2 in range(n_dcol // 2):
                        c_step(1, dc2,
                               stage_next=(dc2 + 1 if dc2 + 1 < n_dcol // 2
                                           else None))

    nc.compile()
    return nc


def mask_kbcnt(mask):
    """kbcnt[b][gqt] from the bool mask [B, S, S] (general, not just
    tril): number of 128-wide key blocks up to the last unmasked key."""
    B, S, _ = mask.shape
    tab = []
    for b in range(B):
        row = []
        for qt in range(S // P):
            m = mask[b, qt * P:(qt + 1) * P, :]
            anyk = np.nonzero(m.any(axis=0))[0]
            last = int(anyk[-1]) if len(anyk) else 0
            row.append(last // P + 1)
        tab.append(row)
    return tab


def shard_inputs(hidden_q, hidden_kv, attention_mask, position_bias,
                 Wq, Wk, Wv, Wo, n_cores=8):
    hidden_q = np.asarray(hidden_q, np.float32)
    hidden_kv = np.asarray(hidden_kv, np.float32)
    attention_mask = np.asarray(attention_mask, bool)
    position_bias = np.asarray(position_bias, np.float32)
    Wq = np.asarray(Wq, np.float32)
    Wk = np.asarray(Wk, np.float32)
    Wv = np.asarray(Wv, np.float32)
    Wo = np.asarray(Wo, np.float32)

    B, S, D = hidden_q.shape
    H = position_bias.shape[1]
    HPC = H // n_cores
    WPC = HPC * DH
    scale = np.float32(1.0 / np.sqrt(DH))

    bf = ml_dtypes.bfloat16
    xq = np.ascontiguousarray(hidden_q.reshape(B * S, D).T).astype(bf)
    xk = np.ascontiguousarray(hidden_kv.reshape(B * S, D).T).astype(bf)
    # ebT[b,h,k,q] = exp(position_bias)*mask, transposed on (q,k)
    ebT = (np.exp(position_bias)
           * attention_mask[:, None, :, :]).transpose(0, 1, 3, 2)
    Wq_s = Wq * scale

    in_maps = []
    for c in range(n_cores):
        sl = slice(c * WPC, (c + 1) * WPC)
        in_maps.append({
            "xqT": xq,
            "xkT": xk,
            "wq": np.ascontiguousarray(Wq_s[:, sl]).astype(bf),
            "wk": np.ascontiguousarray(Wk[:, sl]).astype(bf),
            "wv": np.ascontiguousarray(Wv[:, sl]).astype(bf),
            "wo": np.ascontiguousarray(Wo[sl, :]).astype(bf),
            "ebm": np.ascontiguousarray(
                ebT[:, c * HPC:(c + 1) * HPC]).astype(bf),
        })
    meta = dict(B=B, S=S, D=D, HPC=HPC, nkb_tab=mask_kbcnt(attention_mask))
    return in_maps, meta


_PROG_CACHE = {}


def _get_program(B, S, D, HPC, nkb_key, n_cores):
    key = (B, S, D, HPC, nkb_key, n_cores)
    if key not in _PROG_CACHE:
        _PROG_CACHE[key] = build_program(
            B, S, D, HPC, [list(r) for r in nkb_key], n_cores)
    return _PROG_CACHE[key]


def kernel(hidden_q, hidden_kv, attention_mask, position_bias,
           Wq, Wk, Wv, Wo):
    n_cores = 8
    in_maps, meta = shard_inputs(hidden_q, hidden_kv, attention_mask,
                                 position_bias, Wq, Wk, Wv, Wo, n_cores)
    nkb_key = tuple(tuple(r) for r in meta["nkb_tab"])
    nc = _get_program(meta["B"], meta["S"], meta["D"], meta["HPC"],
                      nkb_key, n_cores)

    from concourse.bass_utils import run_bass_kernel_spmd
    res = None
    for attempt in range(3):
        try:
            res = run_bass_kernel_spmd(nc, in_maps, list(range(n_cores)))
            break
        except Exception:
            # Transient NRT_EXEC_UNIT_UNRECOVERABLE wedges recover on a
            # fresh PJRT client; reset backends and retry.
            if attempt == 2:
                raise
            try:
                import time as _time

                import jax as _jax
                _jax.clear_caches()
                _jax.extend.backend.clear_backends()
                _time.sleep(15 * (attempt + 1))
            except Exception:
                pass

    B, S, D = meta["B"], meta["S"], meta["D"]
    acc = np.zeros((B * S, D), np.float32)
    for r in res.results:
        acc += np.asarray(r["out"], np.float32)
    return acc.reshape(B, S, D)
